# revision 40
# baseline (speedup 1.0000x reference)
"""Trainium2 kernel for nn_DigitConvolutionalModel (dense_cnn).

Model: x[B,784] -> 3x3 valid conv (single channel) -> flatten[676]
       -> Linear(676,200) + ReLU -> Linear(200,10).

The conv is linear, so it is folded into the first Linear on the host:
  flat = x @ C  (C [784,676] sparse conv matrix)
  h1   = relu(flat @ W1.T + b1) = relu(x @ (C @ W1.T) + b1)
so the device computes a plain 784 -> 200 -> 10 MLP. Pure data
parallelism: batch 32768 is split into 8 shards of 4096, one per core;
weights are replicated. Each core receives x pre-transposed ([784,4096],
pixel on the partition/contraction axis) so both matmuls need no
on-device transpose:
  FC1: h1T[200,b] = W1eff[784,200].T @ xT[784,b]   (lhsT = W1eff)
  FC2: outT[10,b] = W2T[200,10].T  @ h1T[200,b]    (lhsT = W2.T)
"""

import os
import numpy as np
from contextlib import ExitStack

import concourse.bass as bass
import concourse.bacc as bacc
import concourse.mybir as mybir
import concourse.tile as tile
from concourse.bass_utils import run_bass_kernel_spmd

import ml_dtypes

N_CORES = 8
B = 32768
BS = B // N_CORES          # 4096 rows per core
IMG = 28
KSZ = 3
OUTW = IMG - KSZ + 1       # 26
NPIX = IMG * IMG           # 784
HID = 200
NCLS = 10

P = 128                    # SBUF partitions
LO_C = 64                  # partition chunk per HWDGE ring
FD = 512                   # matmul free dim (ISA max moving elements; 1 PSUM bank)
NK = 7                     # contraction tiles over 784 = 6*128 + 16
KT = [P] * 6 + [NPIX - 6 * P]
MT = [P, HID - P]          # hid output tiles: 128 + 72
NHALF = 2                  # batch halves per core (PSUM: 2m x 4n = 8 banks)
HB = BS // NHALF           # 2048
NT = HB // FD              # 4 n-tiles of 512 per half

_cache: dict = {}


def _ensure_axon_hooks():
    """Provide antenv.axon_hooks if the image lacks it.

    bass_utils' trace path does `from antenv.axon_hooks import
    get_axon_ntff_profile_hook`; on images without that module the import
    crashes instead of degrading. Register a minimal equivalent that drives
    NTFF profiling via the documented C ABI of the loaded axon PJRT plugin
    (axon_start_nrt_profile / axon_stop_nrt_profile), or returns None so
    bass_utils skips tracing gracefully.
    """
    try:
        import antenv.axon_hooks  # noqa: F401

        return
    except ImportError:
        pass
    import sys
    import types
    import ctypes
    import contextlib

    try:
        import antenv
    except ImportError:
        antenv = types.ModuleType("antenv")
        sys.modules["antenv"] = antenv

    mod = types.ModuleType("antenv.axon_hooks")
    state = {"hook": None, "built": False}

    def _build():
        so_path = None
        try:
            with open("/proc/self/maps") as f:
                for line in f:
                    if "libaxon_pjrt.so" in line:
                        so_path = line.split()[-1]
                        break
        except OSError:
            return None
        if so_path is None:
            return None
        lib = ctypes.CDLL(so_path)
        if not hasattr(lib, "axon_start_nrt_profile"):
            return None
        lib.axon_start_nrt_profile.argtypes = [
            ctypes.POINTER(ctypes.c_int64),
            ctypes.c_size_t,
        ]
        lib.axon_start_nrt_profile.restype = ctypes.c_int64
        lib.axon_stop_nrt_profile.argtypes = [ctypes.c_char_p]
        lib.axon_stop_nrt_profile.restype = ctypes.c_int64

        @contextlib.contextmanager
        def _hook(output_dir, device_ids):
            import jax

            jax.devices()
            if device_ids:
                ids = (ctypes.c_int64 * len(device_ids))(*device_ids)
                rc = lib.axon_start_nrt_profile(ids, len(device_ids))
            else:
                rc = lib.axon_start_nrt_profile(None, 0)
            if rc != 0:
                raise RuntimeError(f"axon_start_nrt_profile rc={rc}")
            try:
                yield
            finally:
                n = lib.axon_stop_nrt_profile(str(output_dir).encode())
                if n <= 0:
                    print(f"ntff profile: rc={n} (no profile written)")

        return _hook

    def get_axon_ntff_profile_hook():
        if not state["built"]:
            state["hook"] = _build()
            state["built"] = True
        return state["hook"]

    def set_axon_ntff_profile_hook(hook):
        state["hook"] = hook
        state["built"] = True

    mod.get_axon_ntff_profile_hook = get_axon_ntff_profile_hook
    mod.set_axon_ntff_profile_hook = set_axon_ntff_profile_hook
    sys.modules["antenv.axon_hooks"] = mod
    antenv.axon_hooks = mod


def _dtypes():
    if os.environ.get("KERNEL_FP32"):
        return mybir.dt.float32, np.float32
    return mybir.dt.bfloat16, ml_dtypes.bfloat16


def _build_nc():
    mm_dt, _ = _dtypes()
    f32 = mybir.dt.float32
    # Bacc (not plain Bass): its compile() pass splits multi-sem waits into
    # standalone EventSemaphore instructions — the TPB ISA allows only one
    # embedded wait per instruction.
    nc = bacc.Bacc(
        "TRN2",
        target_bir_lowering=False,
        debug=False,
        num_devices=N_CORES,
    )

    xT = nc.dram_tensor("xT", [NPIX, BS], mm_dt, kind="ExternalInput")
    w1 = nc.dram_tensor("w1t", [P, NK * HID], mm_dt, kind="ExternalInput")
    w2 = nc.dram_tensor("w2t", [P, 2 * NCLS], mm_dt, kind="ExternalInput")
    b1 = nc.dram_tensor("b1t", [P, 2], f32, kind="ExternalInput")
    b2 = nc.dram_tensor("b2t", [NCLS, 1], f32, kind="ExternalInput")
    outT = nc.dram_tensor("outT", [NCLS, BS], f32, kind="ExternalOutput")

    with ExitStack() as ctx:
        tc = ctx.enter_context(tile.TileContext(nc))
        const = ctx.enter_context(tc.tile_pool(name="const", bufs=1))
        xp = ctx.enter_context(tc.tile_pool(name="xp", bufs=NK))
        h1p = ctx.enter_context(tc.tile_pool(name="h1p", bufs=2))
        op = ctx.enter_context(tc.tile_pool(name="op", bufs=NHALF * NT))
        pp = ctx.enter_context(tc.tile_pool(name="pp", bufs=8, space="PSUM"))

        w1s = const.tile([P, NK * HID], mm_dt)
        w2s = const.tile([P, 2 * NCLS], mm_dt)
        b1s = const.tile([P, 2], f32)
        b2s = const.tile([NCLS, 1], f32)
        # Constants go over SWDGE (gpsimd) so the two HWDGE rings (sync,
        # scalar) are dedicated to the bulk x stream.
        nc.gpsimd.dma_start(w1s[:], w1[:])
        nc.gpsimd.dma_start(w2s[:], w2[:])
        nc.gpsimd.dma_start(b1s[:], b1[:])
        nc.gpsimd.dma_start(b2s[:], b2[:])

        # Load all of x up front: full-width k-tiles (8KB contiguous rows =
        # 8KB DMA descriptors), each tile split into two partition-halves
        # issued on the two independent HWDGE descriptor generators.
        xts = []
        for k in range(NK):
            kt = KT[k]
            xt = xp.tile([P, BS], mm_dt, tag="xt", name=f"xt_{k}")
            lo = min(64, kt)
            nc.sync.dma_start(xt[:lo, :], xT[k * P : k * P + lo, :])
            if kt > lo:
                nc.scalar.dma_start(
                    xt[lo:kt, :], xT[k * P + lo : k * P + kt, :]
                )
            xts.append(xt)

        h1tiles = []
        for h in range(NHALF):
            c0 = h * HB
            ps = [
                [
                    pp.tile([MT[m], FD], f32, tag="bank", name=f"ps_{h}_{m}_{n}")
                    for n in range(NT)
                ]
                for m in range(2)
            ]
            for k in range(NK):
                kt = KT[k]
                for m in range(2):
                    lhsT = w1s[0:kt, k * HID + m * P : k * HID + m * P + MT[m]]
                    for n in range(NT):
                        nc.tensor.matmul(
                            ps[m][n][:],
                            lhsT,
                            xts[k][0:kt, c0 + n * FD : c0 + (n + 1) * FD],
                            start=(k == 0),
                            stop=(k == NK - 1),
                        )
            h1 = [
                h1p.tile([MT[0], HB], mm_dt, tag="h1a", name=f"h1a_{h}"),
                h1p.tile([MT[1], HB], mm_dt, tag="h1b", name=f"h1b_{h}"),
            ]
            # Drains split across ACT (m0, relu via LUT with bias) and DVE
            # (m1, add-bias then max-0) so the banks free twice as fast.
            for n in range(NT):
                nc.scalar.activation(
                    h1[0][:, n * FD : (n + 1) * FD],
                    ps[0][n][:],
                    mybir.ActivationFunctionType.Relu,
                    bias=b1s[0 : MT[0], 0:1],
                )
            for n in range(NT):
                nc.vector.tensor_scalar(
                    h1[1][:, n * FD : (n + 1) * FD],
                    ps[1][n][:],
                    b1s[0 : MT[1], 1:2],
                    0.0,
                    mybir.AluOpType.add,
                    mybir.AluOpType.max,
                )
            h1tiles.append(h1)

        # FC2 emitted after all FC1 matmuls: PE stays dense through FC1,
        # FC2 runs at the tail when h1 has long been drained.
        for h in range(NHALF):
            c0 = h * HB
            h1 = h1tiles[h]
            for n in range(NT):
                ps2 = pp.tile([NCLS, FD], f32, tag="bank", name=f"ps2_{h}_{n}")
                for m in range(2):
                    nc.tensor.matmul(
                        ps2[:],
                        w2s[0 : MT[m], m * NCLS : (m + 1) * NCLS],
                        h1[m][:, n * FD : (n + 1) * FD],
                        start=(m == 0),
                        stop=(m == 1),
                    )
                ot = op.tile([NCLS, FD], f32, tag="ot", name=f"ot_{h}_{n}")
                nc.vector.tensor_scalar_add(ot[:], ps2[:], b2s[:])
                nc.gpsimd.dma_start(
                    outT[:, c0 + n * FD : c0 + (n + 1) * FD], ot[:]
                )

    nc.compile()
    nc.finalize()
    return nc


def _build_nc_raw():
    """Hand-scheduled version (no TileContext): explicit per-engine streams
    and semaphores. Avoids Tile's prologue/epilogue barriers (~13us fixed)."""
    mm_dt, _ = _dtypes()
    f32 = mybir.dt.float32
    nc = bacc.Bacc(
        "TRN2",
        target_bir_lowering=False,
        debug=False,
        num_devices=N_CORES,
    )

    # x arrives pre-arranged on the host as exactly the DMA pieces the
    # kernel issues: xp[g, h, c] is one fully CONTIGUOUS 0.5MB block with
    # 8KB rows = [k=2g | k=2g+1] columns of batch-half h for partition
    # chunk c (rows c*64..). Contiguous source + 8KB rows keeps the HWDGE
    # descriptor pipeline at full rate; the h-ordering lets phase-0
    # compute start long before all of x is resident. k=6 tail separate.
    NG = 3
    xp = nc.dram_tensor(
        "xp", [NG, NHALF, 2, LO_C, 2 * HB], mm_dt, kind="ExternalInput"
    )
    x6 = nc.dram_tensor("x6", [NHALF, KT[6], HB], mm_dt, kind="ExternalInput")
    w1 = nc.dram_tensor("w1t", [P, NK * HID], mm_dt, kind="ExternalInput")
    w2 = nc.dram_tensor("w2t", [P, 2 * NCLS], mm_dt, kind="ExternalInput")
    b1 = nc.dram_tensor("b1t", [P, 2], f32, kind="ExternalInput")
    b2 = nc.dram_tensor("b2t", [NCLS, 1], f32, kind="ExternalInput")
    outT = nc.dram_tensor("outT", [NCLS, BS], f32, kind="ExternalOutput")

    NB = NHALF * NT  # output column blocks of FD

    # SBUF: one tensor per k-tile pair (+ the k=6 tail); pair tensor g holds
    # [k2g-h0 | k2g+1-h0 | k2g-h1 | k2g+1-h1] as 2048-column panels.
    xps = [nc.alloc_sbuf_tensor(f"xp{g}", [P, 2 * BS], mm_dt).ap() for g in range(NG)]
    x6s = nc.alloc_sbuf_tensor("x6s", [KT[6], BS], mm_dt).ap()

    def x_rhs(h, k, n):
        """rhs AP for FC1 matmul (phase h, contraction tile k, n-tile n)."""
        if k == NK - 1:
            return x6s[0 : KT[6], h * HB + n * FD : h * HB + (n + 1) * FD]
        g, half = divmod(k, 2)
        c = (h * 2 + half) * HB + n * FD
        return xps[g][:, c : c + FD]
    w1s = nc.alloc_sbuf_tensor("w1s", [P, NK * HID], mm_dt).ap()
    w2s = nc.alloc_sbuf_tensor("w2s", [P, 2 * NCLS], mm_dt).ap()
    b1s = nc.alloc_sbuf_tensor("b1s", [P, 2], f32).ap()
    b2s = nc.alloc_sbuf_tensor("b2s", [NCLS, 1], f32).ap()
    h1a = nc.alloc_sbuf_tensor("h1a", [MT[0], BS], mm_dt).ap()
    h1b = nc.alloc_sbuf_tensor("h1b", [MT[1], BS], mm_dt).ap()
    ot = nc.alloc_sbuf_tensor("ot", [NCLS, BS], f32).ap()

    # PSUM: 2m x NT tensors covering all 8 banks; FC2 reuses them.
    ps = [
        [nc.alloc_psum_tensor(f"ps_{m}_{n}", [MT[m], FD], f32).ap() for n in range(NT)]
        for m in range(2)
    ]
    ps_flat = [ps[0][n] for n in range(NT)] + [ps[1][n] for n in range(NT)]

    # Each x k-tile is fetched as 4 pieces: partition chunk A (rows 0:64,
    # sync HWDGE ring) / B (rows 64:, scalar ring) x column half h. The
    # column halves let phase-0 compute start long before all of x is in.
    LO = 64

    if True:
        # One sem per transfer (completions across a queue are unordered).
        # alloc_semaphore (not the ctx-manager nc.semaphore) — the ctx exit
        # emits a per-sem clear instruction; we do one range-clear instead.
        s_x = [
            [[nc.alloc_semaphore(f"s_x{c}_{h}_{g}") for g in range(NG)]
             for h in range(NHALF)]
            for c in range(2)
        ]
        s_x6 = [nc.alloc_semaphore(f"s_x6_{h}") for h in range(NHALF)]
        s_w1a = nc.alloc_semaphore("s_w1a")
        s_w1b = nc.alloc_semaphore("s_w1b")
        s_w1a2 = nc.alloc_semaphore("s_w1a2")
        s_w1b2 = nc.alloc_semaphore("s_w1b2")
        s_b1 = nc.alloc_semaphore("s_b1")
        s_b2 = nc.alloc_semaphore("s_b2")
        s_w2 = nc.alloc_semaphore("s_w2")
        s_mm = nc.alloc_semaphore("s_mm")
        s_da = nc.alloc_semaphore("s_da")
        s_dv = nc.alloc_semaphore("s_dv")
        s_f2 = nc.alloc_semaphore("s_f2")
        s_out = nc.alloc_semaphore("s_out")
        s_out2 = nc.alloc_semaphore("s_out2")
        all_sems = (
            [s for c in s_x for h in c for s in h]
            + s_x6
            + [s_w1a, s_w1b, s_w1a2, s_w1b2, s_b1, s_b2, s_w2, s_mm, s_da,
               s_dv, s_f2, s_out, s_out2]
        )

        # closer index (1-based s_mm value) of the last matmul into bank (h,m,n)
        def closer(h, m, n):
            return h * 2 * NT + m * NT + n + 1

        with nc.Block(no_gpsimd_drain=True) as block:

            @block.sync
            def _(sync):
                # k=0/1 weight slices first so the first matmuls start early;
                # the rest of w1 follows the first x pair.
                sync.dma_start(w1s[0:LO, 0 : 2 * HID], w1[0:LO, 0 : 2 * HID]).then_inc(
                    s_w1a, 16
                )
                for h in range(NHALF):
                    c0 = h * 2 * HB
                    for g in range(NG):
                        sync.dma_start(
                            xps[g][0:LO, c0 : c0 + 2 * HB],
                            xp[g, h, 0],
                        ).then_inc(s_x[0][h][g], 16)
                        if h == 0 and g == 0:
                            sync.dma_start(
                                w1s[0:LO, 2 * HID :], w1[0:LO, 2 * HID :]
                            ).then_inc(s_w1a2, 16)
                    sync.dma_start(
                        x6s[:, h * HB : (h + 1) * HB],
                        x6[h],
                    ).then_inc(s_x6[h], 16)
                # second half of the output overlaps the first (other ring)
                sync.wait_ge(s_f2, NB)
                sync.dma_start(
                    outT[:, BS // 2 :], ot[:, BS // 2 :]
                ).then_inc(s_out, 16)
                sync.wait_ge(s_out, 16)

            @block.scalar
            def _(scalar):
                scalar.dma_start(w1s[LO:P, 0 : 2 * HID], w1[LO:P, 0 : 2 * HID]).then_inc(
                    s_w1b, 16
                )
                for h in range(NHALF):
                    c0 = h * 2 * HB
                    for g in range(NG):
                        scalar.dma_start(
                            xps[g][LO:P, c0 : c0 + 2 * HB],
                            xp[g, h, 1],
                        ).then_inc(s_x[1][h][g], 16)
                        if h == 0 and g == 0:
                            scalar.dma_start(
                                w1s[LO:P, 2 * HID :], w1[LO:P, 2 * HID :]
                            ).then_inc(s_w1b2, 16)
                    if h == 0:
                        # biases ride after the phase-0 stream; needed by the
                        # first drains which only start once phase 0 closes
                        scalar.dma_start(b1s[:], b1[:]).then_inc(s_b1, 16)
                scalar.dma_start(b2s[:], b2[:]).then_inc(s_b2, 16)
                scalar.dma_start(w2s[:], w2[:]).then_inc(s_w2, 16)
                # FC1 m0 drains: relu + bias from PSUM -> h1a (bf16 cast)
                scalar.wait_ge(s_b1, 16)  # b1s loaded
                for h in range(NHALF):
                    for n in range(NT):
                        scalar.wait_ge(s_mm, closer(h, 0, n))
                        c = h * HB + n * FD
                        nc.scalar.activation(
                            h1a[:, c : c + FD],
                            ps[0][n][:],
                            mybir.ActivationFunctionType.Relu,
                            bias=b1s[0 : MT[0], 0:1],
                        ).then_inc(s_da, 1)
                # first half of the output, overlapping FC2's second half
                scalar.wait_ge(s_f2, NB // 2)
                scalar.dma_start(
                    outT[:, : BS // 2], ot[:, : BS // 2]
                ).then_inc(s_out2, 16)
                scalar.wait_ge(s_out2, 16)

            @block.tensor
            def _(tensor):
                tensor.wait_ge(s_w1a, 16)
                tensor.wait_ge(s_w1b, 16)
                for h in range(NHALF):
                    for k in range(NK):
                        kt = KT[k]
                        if k == 2:
                            # remainder of w1 (k>=2 column blocks)
                            tensor.wait_ge(s_w1a2, 16)
                            tensor.wait_ge(s_w1b2, 16)
                        if k == NK - 1:
                            tensor.wait_ge(s_x6[h], 16)
                        elif k % 2 == 0:
                            g = k // 2
                            tensor.wait_ge(s_x[0][h][g], 16)
                            tensor.wait_ge(s_x[1][h][g], 16)
                        for m in range(2):
                            lhsT = w1s[0:kt, k * HID + m * P : k * HID + m * P + MT[m]]
                            for n in range(NT):
                                if h == 1 and k == 0:
                                    # bank reuse: wait for phase-0 drain (WAR)
                                    if m == 0:
                                        tensor.wait_ge(s_da, n + 1)
                                    else:
                                        tensor.wait_ge(s_dv, n + 1)
                                mm = nc.tensor.matmul(
                                    ps[m][n][:],
                                    lhsT,
                                    x_rhs(h, k, n),
                                    start=(k == 0),
                                    stop=(k == NK - 1),
                                )
                                if k == NK - 1:
                                    mm.then_inc(s_mm, 1)
                # FC2: block b covers columns [b*FD, (b+1)*FD), reuses ps_flat[b]
                tensor.wait_ge(s_w2, 16)  # w2s loaded
                for b in range(NB):
                    h, n = divmod(b, NT)
                    # bank ps_flat[b] free after its phase-1 FC1 drain
                    if b < NT:
                        tensor.wait_ge(s_da, NT + n + 1)
                    else:
                        tensor.wait_ge(s_dv, NT + n + 1)
                    # h1 slices for this block drained
                    tensor.wait_ge(s_da, b + 1)
                    tensor.wait_ge(s_dv, b + 1)
                    c = b * FD
                    nc.tensor.matmul(
                        ps_flat[b][0:NCLS, :],
                        w2s[0 : MT[0], 0:NCLS],
                        h1a[:, c : c + FD],
                        start=True,
                        stop=False,
                    )
                    nc.tensor.matmul(
                        ps_flat[b][0:NCLS, :],
                        w2s[0 : MT[1], NCLS : 2 * NCLS],
                        h1b[:, c : c + FD],
                        start=False,
                        stop=True,
                    ).then_inc(s_mm, 1)

            @block.vector
            def _(vector):
                vector.wait_ge(s_b1, 16)
                vector.wait_ge(s_b2, 16)
                # FC1 m1 drains: (psum + b1) max 0 -> h1b (bf16 cast)
                for h in range(NHALF):
                    for n in range(NT):
                        vector.wait_ge(s_mm, closer(h, 1, n))
                        c = h * HB + n * FD
                        nc.vector.tensor_scalar(
                            h1b[:, c : c + FD],
                            ps[1][n][:],
                            b1s[0 : MT[1], 1:2],
                            0.0,
                            mybir.AluOpType.add,
                            mybir.AluOpType.max,
                        ).then_inc(s_dv, 1)
                # FC2 drains: psum + b2 -> ot (fp32)
                for b in range(NB):
                    vector.wait_ge(s_mm, 2 * 2 * NT + b + 1)
                    nc.vector.tensor_scalar_add(
                        ot[:, b * FD : (b + 1) * FD], ps_flat[b][0:NCLS, :], b2s[:]
                    ).then_inc(s_f2, 1)

        # After the block-exit all-engine barrier every engine is synced;
        # reset sems so a re-execution of the NEFF starts clean.
        ids = sorted(s.num for s in all_sems)
        if ids == list(range(ids[0], ids[-1] + 1)):
            nc.gpsimd.sem_clear(range(ids[0], ids[-1] + 1))
        else:
            for s in all_sems:
                nc.gpsimd.sem_clear(s)

    nc.compile()
    nc.finalize()
    return nc


def _fold_weights(conv_w, W1):
    """W1eff[784,200] such that x @ W1eff == conv2d_valid(x, conv_w).flat @ W1.T"""
    W1r = W1.reshape(HID, OUTW, OUTW).transpose(1, 2, 0)  # [26,26,200]
    w1e = np.zeros((IMG, IMG, HID), np.float32)
    for di in range(KSZ):
        for dj in range(KSZ):
            w1e[di : di + OUTW, dj : dj + OUTW, :] += conv_w[di, dj] * W1r
    return w1e.reshape(NPIX, HID)


def _prepare_maps(x, conv_w, W1, b1, W2, b2, impl="raw"):
    _, np_dt = _dtypes()
    x = np.asarray(x, np.float32)
    conv_w = np.asarray(conv_w, np.float32)
    W1 = np.asarray(W1, np.float32)
    b1 = np.asarray(b1, np.float32)
    W2 = np.asarray(W2, np.float32)
    b2 = np.asarray(b2, np.float32)

    w1e = _fold_weights(conv_w, W1)
    w1t = np.zeros((P, NK * HID), np_dt)
    for k in range(NK):
        kt = KT[k]
        w1t[:kt, k * HID : (k + 1) * HID] = w1e[k * P : k * P + kt, :].astype(np_dt)
    W2T = W2.T  # [200, 10]
    w2t = np.zeros((P, 2 * NCLS), np_dt)
    w2t[: MT[0], 0:NCLS] = W2T[:P].astype(np_dt)
    w2t[: MT[1], NCLS : 2 * NCLS] = W2T[P:].astype(np_dt)
    b1t = np.zeros((P, 2), np.float32)
    b1t[: MT[0], 0] = b1[:P]
    b1t[: MT[1], 1] = b1[P:]
    b2t = b2.reshape(NCLS, 1)

    xs = x.reshape(N_CORES, BS, NPIX)
    maps = []
    for i in range(N_CORES):
        xTi = xs[i].T.astype(np_dt)  # [784, 4096]
        m = {"w1t": w1t, "w2t": w2t, "b1t": b1t, "b2t": b2t}
        if impl == "raw":
            # exact DMA-piece layout: [group, col-half, partition-chunk]
            # each piece [64, 2*HB] contiguous with 8KB rows [k2g | k2g+1]
            xpv = np.empty((3, NHALF, 2, LO_C, 2 * HB), np_dt)
            for g in range(3):
                r = g * 2 * P
                for h in range(NHALF):
                    cl, ch = h * HB, (h + 1) * HB
                    for c in range(2):
                        r0 = r + c * LO_C
                        xpv[g, h, c, :, 0:HB] = xTi[r0 : r0 + LO_C, cl:ch]
                        xpv[g, h, c, :, HB:] = xTi[P + r0 : P + r0 + LO_C, cl:ch]
            m["xp"] = xpv
            x6v = np.empty((NHALF, NPIX - 6 * P, HB), np_dt)
            for h in range(NHALF):
                x6v[h] = xTi[6 * P :, h * HB : (h + 1) * HB]
            m["x6"] = x6v
        else:
            m["xT"] = xTi
        maps.append(m)
    return maps


def _run(inputs, trace=False):
    _ensure_axon_hooks()
    impl = os.environ.get("KERNEL_IMPL", "raw")
    key = ("nc", impl, bool(os.environ.get("KERNEL_FP32")))
    if key not in _cache:
        _cache[key] = _build_nc_raw() if impl == "raw" else _build_nc()
    nc = _cache[key]
    in_maps = _prepare_maps(**inputs, impl=impl)
    res = run_bass_kernel_spmd(nc, in_maps, list(range(N_CORES)), trace=trace)
    out = np.concatenate([r["outT"].T for r in res.results], axis=0)
    return out, res


def kernel(**inputs):
    out, _ = _run(inputs, trace=False)
    return out


# revision 41
# speedup vs baseline: 1.2490x; 1.2490x over previous
"""Trainium2 kernel for nn_DigitConvolutionalModel (dense_cnn).

Model: x[B,784] -> 3x3 valid conv (single channel) -> flatten[676]
       -> Linear(676,200) + ReLU -> Linear(200,10).

The conv is linear, so it is folded into the first Linear on the host:
  flat = x @ C  (C [784,676] sparse conv matrix)
  h1   = relu(flat @ W1.T + b1) = relu(x @ (C @ W1.T) + b1)
so the device computes a plain 784 -> 200 -> 10 MLP. Pure data
parallelism: batch 32768 is split into 8 shards of 4096, one per core;
weights are replicated. Each core receives x pre-transposed ([784,4096],
pixel on the partition/contraction axis) so both matmuls need no
on-device transpose:
  FC1: h1T[200,b] = W1eff[784,200].T @ xT[784,b]   (lhsT = W1eff)
  FC2: outT[10,b] = W2T[200,10].T  @ h1T[200,b]    (lhsT = W2.T)
"""

import os
import numpy as np
from contextlib import ExitStack

import concourse.bass as bass
import concourse.bacc as bacc
import concourse.mybir as mybir
import concourse.tile as tile
from concourse.bass_utils import run_bass_kernel_spmd

import ml_dtypes

N_CORES = 8
B = 32768
BS = B // N_CORES          # 4096 rows per core
IMG = 28
KSZ = 3
OUTW = IMG - KSZ + 1       # 26
NPIX = IMG * IMG           # 784
HID = 200
NCLS = 10

P = 128                    # SBUF partitions
LO_C = 64                  # partition chunk per HWDGE ring
FD = 512                   # matmul free dim (ISA max moving elements; 1 PSUM bank)
NK = 7                     # contraction tiles over 784 = 6*128 + 16
KT = [P] * 6 + [NPIX - 6 * P]
MT = [P, HID - P]          # hid output tiles: 128 + 72
NHALF = 2                  # batch halves per core (PSUM: 2m x 4n = 8 banks)
HB = BS // NHALF           # 2048
NT = HB // FD              # 4 n-tiles of 512 per half

_cache: dict = {}


def _ensure_axon_hooks():
    """Provide antenv.axon_hooks if the image lacks it.

    bass_utils' trace path does `from antenv.axon_hooks import
    get_axon_ntff_profile_hook`; on images without that module the import
    crashes instead of degrading. Register a minimal equivalent that drives
    NTFF profiling via the documented C ABI of the loaded axon PJRT plugin
    (axon_start_nrt_profile / axon_stop_nrt_profile), or returns None so
    bass_utils skips tracing gracefully.
    """
    try:
        import antenv.axon_hooks  # noqa: F401

        return
    except ImportError:
        pass
    import sys
    import types
    import ctypes
    import contextlib

    try:
        import antenv
    except ImportError:
        antenv = types.ModuleType("antenv")
        sys.modules["antenv"] = antenv

    mod = types.ModuleType("antenv.axon_hooks")
    state = {"hook": None, "built": False}

    def _build():
        so_path = None
        try:
            with open("/proc/self/maps") as f:
                for line in f:
                    if "libaxon_pjrt.so" in line:
                        so_path = line.split()[-1]
                        break
        except OSError:
            return None
        if so_path is None:
            return None
        lib = ctypes.CDLL(so_path)
        if not hasattr(lib, "axon_start_nrt_profile"):
            return None
        lib.axon_start_nrt_profile.argtypes = [
            ctypes.POINTER(ctypes.c_int64),
            ctypes.c_size_t,
        ]
        lib.axon_start_nrt_profile.restype = ctypes.c_int64
        lib.axon_stop_nrt_profile.argtypes = [ctypes.c_char_p]
        lib.axon_stop_nrt_profile.restype = ctypes.c_int64

        @contextlib.contextmanager
        def _hook(output_dir, device_ids):
            import jax

            jax.devices()
            if device_ids:
                ids = (ctypes.c_int64 * len(device_ids))(*device_ids)
                rc = lib.axon_start_nrt_profile(ids, len(device_ids))
            else:
                rc = lib.axon_start_nrt_profile(None, 0)
            if rc != 0:
                raise RuntimeError(f"axon_start_nrt_profile rc={rc}")
            try:
                yield
            finally:
                n = lib.axon_stop_nrt_profile(str(output_dir).encode())
                if n <= 0:
                    print(f"ntff profile: rc={n} (no profile written)")

        return _hook

    def get_axon_ntff_profile_hook():
        if not state["built"]:
            state["hook"] = _build()
            state["built"] = True
        return state["hook"]

    def set_axon_ntff_profile_hook(hook):
        state["hook"] = hook
        state["built"] = True

    mod.get_axon_ntff_profile_hook = get_axon_ntff_profile_hook
    mod.set_axon_ntff_profile_hook = set_axon_ntff_profile_hook
    sys.modules["antenv.axon_hooks"] = mod
    antenv.axon_hooks = mod


def _dtypes():
    if os.environ.get("KERNEL_FP32"):
        return mybir.dt.float32, np.float32
    return mybir.dt.bfloat16, ml_dtypes.bfloat16


def _build_nc():
    mm_dt, _ = _dtypes()
    f32 = mybir.dt.float32
    # Bacc (not plain Bass): its compile() pass splits multi-sem waits into
    # standalone EventSemaphore instructions — the TPB ISA allows only one
    # embedded wait per instruction.
    nc = bacc.Bacc(
        "TRN2",
        target_bir_lowering=False,
        debug=False,
        num_devices=N_CORES,
    )

    xT = nc.dram_tensor("xT", [NPIX, BS], mm_dt, kind="ExternalInput")
    w1 = nc.dram_tensor("w1t", [P, NK * HID], mm_dt, kind="ExternalInput")
    w2 = nc.dram_tensor("w2t", [P, 2 * NCLS], mm_dt, kind="ExternalInput")
    b1 = nc.dram_tensor("b1t", [P, 2], f32, kind="ExternalInput")
    b2 = nc.dram_tensor("b2t", [NCLS, 1], f32, kind="ExternalInput")
    outT = nc.dram_tensor("outT", [NCLS, BS], f32, kind="ExternalOutput")

    with ExitStack() as ctx:
        tc = ctx.enter_context(tile.TileContext(nc))
        const = ctx.enter_context(tc.tile_pool(name="const", bufs=1))
        xp = ctx.enter_context(tc.tile_pool(name="xp", bufs=NHALF * NK))
        h1p = ctx.enter_context(tc.tile_pool(name="h1p", bufs=2))
        op = ctx.enter_context(tc.tile_pool(name="op", bufs=NHALF * NT))
        pp = ctx.enter_context(tc.tile_pool(name="pp", bufs=8, space="PSUM"))

        w1s = const.tile([P, NK * HID], mm_dt)
        w2s = const.tile([P, 2 * NCLS], mm_dt)
        b1s = const.tile([P, 2], f32)
        b2s = const.tile([NCLS, 1], f32)
        nc.sync.dma_start(w1s[:], w1[:])
        nc.sync.dma_start(w2s[:], w2[:])
        nc.sync.dma_start(b1s[:], b1[:])
        nc.sync.dma_start(b2s[:], b2[:])

        h1tiles = []
        for h in range(NHALF):
            c0 = h * HB
            ps = [
                [
                    pp.tile([MT[m], FD], f32, tag="bank", name=f"ps_{h}_{m}_{n}")
                    for n in range(NT)
                ]
                for m in range(2)
            ]
            for k in range(NK):
                kt = KT[k]
                xt = xp.tile([P, HB], mm_dt, tag="xt", name=f"xt_{h}_{k}")
                nc.sync.dma_start(xt[:kt, :], xT[k * P : k * P + kt, c0 : c0 + HB])
                for m in range(2):
                    lhsT = w1s[0:kt, k * HID + m * P : k * HID + m * P + MT[m]]
                    for n in range(NT):
                        nc.tensor.matmul(
                            ps[m][n][:],
                            lhsT,
                            xt[0:kt, n * FD : (n + 1) * FD],
                            start=(k == 0),
                            stop=(k == NK - 1),
                        )
            h1 = [
                h1p.tile([MT[0], HB], mm_dt, tag="h1a", name=f"h1a_{h}"),
                h1p.tile([MT[1], HB], mm_dt, tag="h1b", name=f"h1b_{h}"),
            ]
            # Drains split across ACT (m0, relu via LUT with bias) and DVE
            # (m1, add-bias then max-0) so the banks free twice as fast.
            for n in range(NT):
                nc.scalar.activation(
                    h1[0][:, n * FD : (n + 1) * FD],
                    ps[0][n][:],
                    mybir.ActivationFunctionType.Relu,
                    bias=b1s[0 : MT[0], 0:1],
                )
            for n in range(NT):
                nc.vector.tensor_scalar(
                    h1[1][:, n * FD : (n + 1) * FD],
                    ps[1][n][:],
                    b1s[0 : MT[1], 1:2],
                    0.0,
                    mybir.AluOpType.add,
                    mybir.AluOpType.max,
                )
            h1tiles.append(h1)

        # FC2 emitted after all FC1 matmuls: PE stays dense through FC1,
        # FC2 runs at the tail when h1 has long been drained.
        for h in range(NHALF):
            c0 = h * HB
            h1 = h1tiles[h]
            for n in range(NT):
                ps2 = pp.tile([NCLS, FD], f32, tag="bank", name=f"ps2_{h}_{n}")
                for m in range(2):
                    nc.tensor.matmul(
                        ps2[:],
                        w2s[0 : MT[m], m * NCLS : (m + 1) * NCLS],
                        h1[m][:, n * FD : (n + 1) * FD],
                        start=(m == 0),
                        stop=(m == 1),
                    )
                ot = op.tile([NCLS, FD], f32, tag="ot", name=f"ot_{h}_{n}")
                nc.vector.tensor_scalar_add(ot[:], ps2[:], b2s[:])
                nc.gpsimd.dma_start(
                    outT[:, c0 + n * FD : c0 + (n + 1) * FD], ot[:]
                )

    nc.compile()
    nc.finalize()
    return nc


def _build_nc_raw():
    """Hand-scheduled version (no TileContext): explicit per-engine streams
    and semaphores. Avoids Tile's prologue/epilogue barriers (~13us fixed)."""
    mm_dt, _ = _dtypes()
    f32 = mybir.dt.float32
    nc = bacc.Bacc(
        "TRN2",
        target_bir_lowering=False,
        debug=False,
        num_devices=N_CORES,
    )

    # x arrives pre-arranged on the host as exactly the DMA pieces the
    # kernel issues: xp[g, h, c] is one fully CONTIGUOUS 0.5MB block with
    # 8KB rows = [k=2g | k=2g+1] columns of batch-half h for partition
    # chunk c (rows c*64..). Contiguous source + 8KB rows keeps the HWDGE
    # descriptor pipeline at full rate; the h-ordering lets phase-0
    # compute start long before all of x is resident. k=6 tail separate.
    NG = 3
    xp = nc.dram_tensor(
        "xp", [NG, NHALF, 2, LO_C, 2 * HB], mm_dt, kind="ExternalInput"
    )
    x6 = nc.dram_tensor("x6", [NHALF, KT[6], HB], mm_dt, kind="ExternalInput")
    w1 = nc.dram_tensor("w1t", [P, NK * HID], mm_dt, kind="ExternalInput")
    w2 = nc.dram_tensor("w2t", [P, 2 * NCLS], mm_dt, kind="ExternalInput")
    b1 = nc.dram_tensor("b1t", [P, 2], f32, kind="ExternalInput")
    b2 = nc.dram_tensor("b2t", [NCLS, 1], f32, kind="ExternalInput")
    outT = nc.dram_tensor("outT", [NCLS, BS], f32, kind="ExternalOutput")

    NB = NHALF * NT  # output column blocks of FD

    # SBUF: one tensor per k-tile pair (+ the k=6 tail); pair tensor g holds
    # [k2g-h0 | k2g+1-h0 | k2g-h1 | k2g+1-h1] as 2048-column panels.
    xps = [nc.alloc_sbuf_tensor(f"xp{g}", [P, 2 * BS], mm_dt).ap() for g in range(NG)]
    x6s = nc.alloc_sbuf_tensor("x6s", [KT[6], BS], mm_dt).ap()

    def x_rhs(h, k, n):
        """rhs AP for FC1 matmul (phase h, contraction tile k, n-tile n)."""
        if k == NK - 1:
            return x6s[0 : KT[6], h * HB + n * FD : h * HB + (n + 1) * FD]
        g, half = divmod(k, 2)
        c = (h * 2 + half) * HB + n * FD
        return xps[g][:, c : c + FD]
    w1s = nc.alloc_sbuf_tensor("w1s", [P, NK * HID], mm_dt).ap()
    w2s = nc.alloc_sbuf_tensor("w2s", [P, 2 * NCLS], mm_dt).ap()
    b1s = nc.alloc_sbuf_tensor("b1s", [P, 2], f32).ap()
    b2s = nc.alloc_sbuf_tensor("b2s", [NCLS, 1], f32).ap()
    h1a = nc.alloc_sbuf_tensor("h1a", [MT[0], BS], mm_dt).ap()
    h1b = nc.alloc_sbuf_tensor("h1b", [MT[1], BS], mm_dt).ap()
    ot = nc.alloc_sbuf_tensor("ot", [NCLS, BS], f32).ap()

    # PSUM: 2m x NT tensors covering all 8 banks; FC2 reuses them.
    ps = [
        [nc.alloc_psum_tensor(f"ps_{m}_{n}", [MT[m], FD], f32).ap() for n in range(NT)]
        for m in range(2)
    ]
    ps_flat = [ps[0][n] for n in range(NT)] + [ps[1][n] for n in range(NT)]

    # Each x k-tile is fetched as 4 pieces: partition chunk A (rows 0:64,
    # sync HWDGE ring) / B (rows 64:, scalar ring) x column half h. The
    # column halves let phase-0 compute start long before all of x is in.
    LO = 64

    if True:
        # One sem per transfer (completions across a queue are unordered).
        # alloc_semaphore (not the ctx-manager nc.semaphore) — the ctx exit
        # emits a per-sem clear instruction; we do one range-clear instead.
        s_x = [
            [[nc.alloc_semaphore(f"s_x{c}_{h}_{g}") for g in range(NG)]
             for h in range(NHALF)]
            for c in range(2)
        ]
        s_x6 = [nc.alloc_semaphore(f"s_x6_{h}") for h in range(NHALF)]
        s_w1a = nc.alloc_semaphore("s_w1a")
        s_w1b = nc.alloc_semaphore("s_w1b")
        s_w1a2 = nc.alloc_semaphore("s_w1a2")
        s_w1b2 = nc.alloc_semaphore("s_w1b2")
        s_b1 = nc.alloc_semaphore("s_b1")
        s_b2 = nc.alloc_semaphore("s_b2")
        s_w2 = nc.alloc_semaphore("s_w2")
        s_mm = nc.alloc_semaphore("s_mm")
        s_da = nc.alloc_semaphore("s_da")
        s_dv = nc.alloc_semaphore("s_dv")
        s_f2 = nc.alloc_semaphore("s_f2")
        s_out = nc.alloc_semaphore("s_out")
        s_out2 = nc.alloc_semaphore("s_out2")
        all_sems = (
            [s for c in s_x for h in c for s in h]
            + s_x6
            + [s_w1a, s_w1b, s_w1a2, s_w1b2, s_b1, s_b2, s_w2, s_mm, s_da,
               s_dv, s_f2, s_out, s_out2]
        )

        # closer index (1-based s_mm value) of the last matmul into bank (h,m,n)
        def closer(h, m, n):
            return h * 2 * NT + m * NT + n + 1

        with nc.Block(no_gpsimd_drain=True) as block:

            @block.sync
            def _(sync):
                # k=0/1 weight slices first so the first matmuls start early;
                # the rest of w1 follows the first x pair.
                sync.dma_start(w1s[0:LO, 0 : 2 * HID], w1[0:LO, 0 : 2 * HID]).then_inc(
                    s_w1a, 16
                )
                for h in range(NHALF):
                    c0 = h * 2 * HB
                    for g in range(NG):
                        sync.dma_start(
                            xps[g][0:LO, c0 : c0 + 2 * HB],
                            xp[g, h, 0],
                        ).then_inc(s_x[0][h][g], 16)
                        if h == 0 and g == 0:
                            sync.dma_start(
                                w1s[0:LO, 2 * HID :], w1[0:LO, 2 * HID :]
                            ).then_inc(s_w1a2, 16)
                    sync.dma_start(
                        x6s[:, h * HB : (h + 1) * HB],
                        x6[h],
                    ).then_inc(s_x6[h], 16)
                # second half of the output overlaps the first (other ring)
                sync.wait_ge(s_f2, NB)
                sync.dma_start(
                    outT[:, BS // 2 :], ot[:, BS // 2 :]
                ).then_inc(s_out, 16)
                sync.wait_ge(s_out, 16)

            @block.scalar
            def _(scalar):
                scalar.dma_start(w1s[LO:P, 0 : 2 * HID], w1[LO:P, 0 : 2 * HID]).then_inc(
                    s_w1b, 16
                )
                for h in range(NHALF):
                    c0 = h * 2 * HB
                    for g in range(NG):
                        scalar.dma_start(
                            xps[g][LO:P, c0 : c0 + 2 * HB],
                            xp[g, h, 1],
                        ).then_inc(s_x[1][h][g], 16)
                        if h == 0 and g == 0:
                            scalar.dma_start(
                                w1s[LO:P, 2 * HID :], w1[LO:P, 2 * HID :]
                            ).then_inc(s_w1b2, 16)
                    if h == 0:
                        # biases ride after the phase-0 stream; needed by the
                        # first drains which only start once phase 0 closes
                        scalar.dma_start(b1s[:], b1[:]).then_inc(s_b1, 16)
                scalar.dma_start(b2s[:], b2[:]).then_inc(s_b2, 16)
                scalar.dma_start(w2s[:], w2[:]).then_inc(s_w2, 16)
                # FC1 m0 drains: relu + bias from PSUM -> h1a (bf16 cast)
                scalar.wait_ge(s_b1, 16)  # b1s loaded
                for h in range(NHALF):
                    for n in range(NT):
                        scalar.wait_ge(s_mm, closer(h, 0, n))
                        c = h * HB + n * FD
                        nc.scalar.activation(
                            h1a[:, c : c + FD],
                            ps[0][n][:],
                            mybir.ActivationFunctionType.Relu,
                            bias=b1s[0 : MT[0], 0:1],
                        ).then_inc(s_da, 1)
                # first half of the output, overlapping FC2's second half
                scalar.wait_ge(s_f2, NB // 2)
                scalar.dma_start(
                    outT[:, : BS // 2], ot[:, : BS // 2]
                ).then_inc(s_out2, 16)
                scalar.wait_ge(s_out2, 16)

            @block.tensor
            def _(tensor):
                tensor.wait_ge(s_w1a, 16)
                tensor.wait_ge(s_w1b, 16)
                for h in range(NHALF):
                    for k in range(NK):
                        kt = KT[k]
                        if k == 2:
                            # remainder of w1 (k>=2 column blocks)
                            tensor.wait_ge(s_w1a2, 16)
                            tensor.wait_ge(s_w1b2, 16)
                        if k == NK - 1:
                            tensor.wait_ge(s_x6[h], 16)
                        elif k % 2 == 0:
                            g = k // 2
                            tensor.wait_ge(s_x[0][h][g], 16)
                            tensor.wait_ge(s_x[1][h][g], 16)
                        for m in range(2):
                            lhsT = w1s[0:kt, k * HID + m * P : k * HID + m * P + MT[m]]
                            for n in range(NT):
                                if h == 1 and k == 0:
                                    # bank reuse: wait for phase-0 drain (WAR)
                                    if m == 0:
                                        tensor.wait_ge(s_da, n + 1)
                                    else:
                                        tensor.wait_ge(s_dv, n + 1)
                                mm = nc.tensor.matmul(
                                    ps[m][n][:],
                                    lhsT,
                                    x_rhs(h, k, n),
                                    start=(k == 0),
                                    stop=(k == NK - 1),
                                )
                                if k == NK - 1:
                                    mm.then_inc(s_mm, 1)
                # FC2: block b covers columns [b*FD, (b+1)*FD), reuses ps_flat[b]
                tensor.wait_ge(s_w2, 16)  # w2s loaded
                for b in range(NB):
                    h, n = divmod(b, NT)
                    # bank ps_flat[b] free after its phase-1 FC1 drain
                    if b < NT:
                        tensor.wait_ge(s_da, NT + n + 1)
                    else:
                        tensor.wait_ge(s_dv, NT + n + 1)
                    # h1 slices for this block drained
                    tensor.wait_ge(s_da, b + 1)
                    tensor.wait_ge(s_dv, b + 1)
                    c = b * FD
                    nc.tensor.matmul(
                        ps_flat[b][0:NCLS, :],
                        w2s[0 : MT[0], 0:NCLS],
                        h1a[:, c : c + FD],
                        start=True,
                        stop=False,
                    )
                    nc.tensor.matmul(
                        ps_flat[b][0:NCLS, :],
                        w2s[0 : MT[1], NCLS : 2 * NCLS],
                        h1b[:, c : c + FD],
                        start=False,
                        stop=True,
                    ).then_inc(s_mm, 1)

            @block.vector
            def _(vector):
                vector.wait_ge(s_b1, 16)
                vector.wait_ge(s_b2, 16)
                # FC1 m1 drains: (psum + b1) max 0 -> h1b (bf16 cast)
                for h in range(NHALF):
                    for n in range(NT):
                        vector.wait_ge(s_mm, closer(h, 1, n))
                        c = h * HB + n * FD
                        nc.vector.tensor_scalar(
                            h1b[:, c : c + FD],
                            ps[1][n][:],
                            b1s[0 : MT[1], 1:2],
                            0.0,
                            mybir.AluOpType.add,
                            mybir.AluOpType.max,
                        ).then_inc(s_dv, 1)
                # FC2 drains: psum + b2 -> ot (fp32)
                for b in range(NB):
                    vector.wait_ge(s_mm, 2 * 2 * NT + b + 1)
                    nc.vector.tensor_scalar_add(
                        ot[:, b * FD : (b + 1) * FD], ps_flat[b][0:NCLS, :], b2s[:]
                    ).then_inc(s_f2, 1)

        # After the block-exit all-engine barrier every engine is synced;
        # reset sems so a re-execution of the NEFF starts clean.
        ids = sorted(s.num for s in all_sems)
        if ids == list(range(ids[0], ids[-1] + 1)):
            nc.gpsimd.sem_clear(range(ids[0], ids[-1] + 1))
        else:
            for s in all_sems:
                nc.gpsimd.sem_clear(s)

    nc.compile()
    nc.finalize()
    return nc


def _fold_weights(conv_w, W1):
    """W1eff[784,200] such that x @ W1eff == conv2d_valid(x, conv_w).flat @ W1.T"""
    W1r = W1.reshape(HID, OUTW, OUTW).transpose(1, 2, 0)  # [26,26,200]
    w1e = np.zeros((IMG, IMG, HID), np.float32)
    for di in range(KSZ):
        for dj in range(KSZ):
            w1e[di : di + OUTW, dj : dj + OUTW, :] += conv_w[di, dj] * W1r
    return w1e.reshape(NPIX, HID)


def _prepare_maps(x, conv_w, W1, b1, W2, b2, impl="raw"):
    _, np_dt = _dtypes()
    x = np.asarray(x, np.float32)
    conv_w = np.asarray(conv_w, np.float32)
    W1 = np.asarray(W1, np.float32)
    b1 = np.asarray(b1, np.float32)
    W2 = np.asarray(W2, np.float32)
    b2 = np.asarray(b2, np.float32)

    w1e = _fold_weights(conv_w, W1)
    w1t = np.zeros((P, NK * HID), np_dt)
    for k in range(NK):
        kt = KT[k]
        w1t[:kt, k * HID : (k + 1) * HID] = w1e[k * P : k * P + kt, :].astype(np_dt)
    W2T = W2.T  # [200, 10]
    w2t = np.zeros((P, 2 * NCLS), np_dt)
    w2t[: MT[0], 0:NCLS] = W2T[:P].astype(np_dt)
    w2t[: MT[1], NCLS : 2 * NCLS] = W2T[P:].astype(np_dt)
    b1t = np.zeros((P, 2), np.float32)
    b1t[: MT[0], 0] = b1[:P]
    b1t[: MT[1], 1] = b1[P:]
    b2t = b2.reshape(NCLS, 1)

    xs = x.reshape(N_CORES, BS, NPIX)
    maps = []
    for i in range(N_CORES):
        xTi = xs[i].T.astype(np_dt)  # [784, 4096]
        m = {"w1t": w1t, "w2t": w2t, "b1t": b1t, "b2t": b2t}
        if impl == "raw":
            # exact DMA-piece layout: [group, col-half, partition-chunk]
            # each piece [64, 2*HB] contiguous with 8KB rows [k2g | k2g+1]
            xpv = np.empty((3, NHALF, 2, LO_C, 2 * HB), np_dt)
            for g in range(3):
                r = g * 2 * P
                for h in range(NHALF):
                    cl, ch = h * HB, (h + 1) * HB
                    for c in range(2):
                        r0 = r + c * LO_C
                        xpv[g, h, c, :, 0:HB] = xTi[r0 : r0 + LO_C, cl:ch]
                        xpv[g, h, c, :, HB:] = xTi[P + r0 : P + r0 + LO_C, cl:ch]
            m["xp"] = xpv
            x6v = np.empty((NHALF, NPIX - 6 * P, HB), np_dt)
            for h in range(NHALF):
                x6v[h] = xTi[6 * P :, h * HB : (h + 1) * HB]
            m["x6"] = x6v
        else:
            m["xT"] = xTi
        maps.append(m)
    return maps


def _run(inputs, trace=False):
    _ensure_axon_hooks()
    impl = os.environ.get("KERNEL_IMPL", "raw")
    key = ("nc", impl, bool(os.environ.get("KERNEL_FP32")))
    if key not in _cache:
        _cache[key] = _build_nc_raw() if impl == "raw" else _build_nc()
    nc = _cache[key]
    in_maps = _prepare_maps(**inputs, impl=impl)
    res = run_bass_kernel_spmd(nc, in_maps, list(range(N_CORES)), trace=trace)
    out = np.concatenate([r["outT"].T for r in res.results], axis=0)
    return out, res


def kernel(**inputs):
    out, _ = _run(inputs, trace=False)
    return out


# revision 42
# speedup vs baseline: 1.2645x; 1.0124x over previous
"""Trainium2 kernel for nn_DigitConvolutionalModel (dense_cnn).

Model: x[B,784] -> 3x3 valid conv (single channel) -> flatten[676]
       -> Linear(676,200) + ReLU -> Linear(200,10).

The conv is linear, so it is folded into the first Linear on the host:
  flat = x @ C  (C [784,676] sparse conv matrix)
  h1   = relu(flat @ W1.T + b1) = relu(x @ (C @ W1.T) + b1)
so the device computes a plain 784 -> 200 -> 10 MLP. Pure data
parallelism: batch 32768 is split into 8 shards of 4096, one per core;
weights are replicated. Each core receives x pre-transposed ([784,4096],
pixel on the partition/contraction axis) so both matmuls need no
on-device transpose:
  FC1: h1T[200,b] = W1eff[784,200].T @ xT[784,b]   (lhsT = W1eff)
  FC2: outT[10,b] = W2T[200,10].T  @ h1T[200,b]    (lhsT = W2.T)
"""

import os
import numpy as np
from contextlib import ExitStack

import concourse.bass as bass
import concourse.bacc as bacc
import concourse.mybir as mybir
import concourse.tile as tile
from concourse.bass_utils import run_bass_kernel_spmd

import ml_dtypes

N_CORES = 8
B = 32768
BS = B // N_CORES          # 4096 rows per core
IMG = 28
KSZ = 3
OUTW = IMG - KSZ + 1       # 26
NPIX = IMG * IMG           # 784
HID = 200
NCLS = 10

P = 128                    # SBUF partitions
LO_C = 64                  # partition chunk per HWDGE ring
FD = 512                   # matmul free dim (ISA max moving elements; 1 PSUM bank)
NK = 7                     # contraction tiles over 784 = 6*128 + 16
KT = [P] * 6 + [NPIX - 6 * P]
MT = [P, HID - P]          # hid output tiles: 128 + 72
NHALF = 2                  # batch halves per core (PSUM: 2m x 4n = 8 banks)
HB = BS // NHALF           # 2048
NT = HB // FD              # 4 n-tiles of 512 per half

_cache: dict = {}


def _ensure_axon_hooks():
    """Provide antenv.axon_hooks if the image lacks it.

    bass_utils' trace path does `from antenv.axon_hooks import
    get_axon_ntff_profile_hook`; on images without that module the import
    crashes instead of degrading. Register a minimal equivalent that drives
    NTFF profiling via the documented C ABI of the loaded axon PJRT plugin
    (axon_start_nrt_profile / axon_stop_nrt_profile), or returns None so
    bass_utils skips tracing gracefully.
    """
    try:
        import antenv.axon_hooks  # noqa: F401

        return
    except ImportError:
        pass
    import sys
    import types
    import ctypes
    import contextlib

    try:
        import antenv
    except ImportError:
        antenv = types.ModuleType("antenv")
        sys.modules["antenv"] = antenv

    mod = types.ModuleType("antenv.axon_hooks")
    state = {"hook": None, "built": False}

    def _build():
        so_path = None
        try:
            with open("/proc/self/maps") as f:
                for line in f:
                    if "libaxon_pjrt.so" in line:
                        so_path = line.split()[-1]
                        break
        except OSError:
            return None
        if so_path is None:
            return None
        lib = ctypes.CDLL(so_path)
        if not hasattr(lib, "axon_start_nrt_profile"):
            return None
        lib.axon_start_nrt_profile.argtypes = [
            ctypes.POINTER(ctypes.c_int64),
            ctypes.c_size_t,
        ]
        lib.axon_start_nrt_profile.restype = ctypes.c_int64
        lib.axon_stop_nrt_profile.argtypes = [ctypes.c_char_p]
        lib.axon_stop_nrt_profile.restype = ctypes.c_int64

        @contextlib.contextmanager
        def _hook(output_dir, device_ids):
            import jax

            jax.devices()
            if device_ids:
                ids = (ctypes.c_int64 * len(device_ids))(*device_ids)
                rc = lib.axon_start_nrt_profile(ids, len(device_ids))
            else:
                rc = lib.axon_start_nrt_profile(None, 0)
            if rc != 0:
                raise RuntimeError(f"axon_start_nrt_profile rc={rc}")
            try:
                yield
            finally:
                n = lib.axon_stop_nrt_profile(str(output_dir).encode())
                if n <= 0:
                    print(f"ntff profile: rc={n} (no profile written)")

        return _hook

    def get_axon_ntff_profile_hook():
        if not state["built"]:
            state["hook"] = _build()
            state["built"] = True
        return state["hook"]

    def set_axon_ntff_profile_hook(hook):
        state["hook"] = hook
        state["built"] = True

    mod.get_axon_ntff_profile_hook = get_axon_ntff_profile_hook
    mod.set_axon_ntff_profile_hook = set_axon_ntff_profile_hook
    sys.modules["antenv.axon_hooks"] = mod
    antenv.axon_hooks = mod


def _dtypes():
    if os.environ.get("KERNEL_FP32"):
        return mybir.dt.float32, np.float32
    return mybir.dt.bfloat16, ml_dtypes.bfloat16


def _build_nc():
    mm_dt, _ = _dtypes()
    f32 = mybir.dt.float32
    # Bacc (not plain Bass): its compile() pass splits multi-sem waits into
    # standalone EventSemaphore instructions — the TPB ISA allows only one
    # embedded wait per instruction.
    nc = bacc.Bacc(
        "TRN2",
        target_bir_lowering=False,
        debug=False,
        num_devices=N_CORES,
    )

    xT = nc.dram_tensor("xT", [NPIX, BS], mm_dt, kind="ExternalInput")
    w1 = nc.dram_tensor("w1t", [P, NK * HID], mm_dt, kind="ExternalInput")
    w2 = nc.dram_tensor("w2t", [P, 2 * NCLS], mm_dt, kind="ExternalInput")
    b1 = nc.dram_tensor("b1t", [P, 2], f32, kind="ExternalInput")
    b2 = nc.dram_tensor("b2t", [NCLS, 1], f32, kind="ExternalInput")
    outT = nc.dram_tensor("outT", [NCLS, BS], f32, kind="ExternalOutput")

    with ExitStack() as ctx:
        tc = ctx.enter_context(tile.TileContext(nc))
        const = ctx.enter_context(tc.tile_pool(name="const", bufs=1))
        xp = ctx.enter_context(tc.tile_pool(name="xp", bufs=NHALF * NK))
        h1p = ctx.enter_context(tc.tile_pool(name="h1p", bufs=2))
        op = ctx.enter_context(tc.tile_pool(name="op", bufs=NHALF * NT))
        pp = ctx.enter_context(tc.tile_pool(name="pp", bufs=8, space="PSUM"))

        w1s = const.tile([P, NK * HID], mm_dt)
        w2s = const.tile([P, 2 * NCLS], mm_dt)
        b1s = const.tile([P, 2], f32)
        b2s = const.tile([NCLS, 1], f32)
        nc.sync.dma_start(w1s[:], w1[:])
        nc.sync.dma_start(w2s[:], w2[:])
        nc.sync.dma_start(b1s[:], b1[:])
        nc.sync.dma_start(b2s[:], b2[:])

        h1tiles = []
        for h in range(NHALF):
            c0 = h * HB
            ps = [
                [
                    pp.tile([MT[m], FD], f32, tag="bank", name=f"ps_{h}_{m}_{n}")
                    for n in range(NT)
                ]
                for m in range(2)
            ]
            for k in range(NK):
                kt = KT[k]
                xt = xp.tile([P, HB], mm_dt, tag="xt", name=f"xt_{h}_{k}")
                nc.sync.dma_start(xt[:kt, :], xT[k * P : k * P + kt, c0 : c0 + HB])
                for m in range(2):
                    lhsT = w1s[0:kt, k * HID + m * P : k * HID + m * P + MT[m]]
                    for n in range(NT):
                        nc.tensor.matmul(
                            ps[m][n][:],
                            lhsT,
                            xt[0:kt, n * FD : (n + 1) * FD],
                            start=(k == 0),
                            stop=(k == NK - 1),
                        )
            h1 = [
                h1p.tile([MT[0], HB], mm_dt, tag="h1a", name=f"h1a_{h}"),
                h1p.tile([MT[1], HB], mm_dt, tag="h1b", name=f"h1b_{h}"),
            ]
            # Drains split across ACT (m0, relu via LUT with bias) and DVE
            # (m1, add-bias then max-0) so the banks free twice as fast.
            for n in range(NT):
                nc.scalar.activation(
                    h1[0][:, n * FD : (n + 1) * FD],
                    ps[0][n][:],
                    mybir.ActivationFunctionType.Relu,
                    bias=b1s[0 : MT[0], 0:1],
                )
            for n in range(NT):
                nc.vector.tensor_scalar(
                    h1[1][:, n * FD : (n + 1) * FD],
                    ps[1][n][:],
                    b1s[0 : MT[1], 1:2],
                    0.0,
                    mybir.AluOpType.add,
                    mybir.AluOpType.max,
                )
            h1tiles.append(h1)

        # FC2 emitted after all FC1 matmuls: PE stays dense through FC1,
        # FC2 runs at the tail when h1 has long been drained.
        for h in range(NHALF):
            c0 = h * HB
            h1 = h1tiles[h]
            for n in range(NT):
                ps2 = pp.tile([NCLS, FD], f32, tag="bank", name=f"ps2_{h}_{n}")
                for m in range(2):
                    nc.tensor.matmul(
                        ps2[:],
                        w2s[0 : MT[m], m * NCLS : (m + 1) * NCLS],
                        h1[m][:, n * FD : (n + 1) * FD],
                        start=(m == 0),
                        stop=(m == 1),
                    )
                ot = op.tile([NCLS, FD], f32, tag="ot", name=f"ot_{h}_{n}")
                nc.vector.tensor_scalar_add(ot[:], ps2[:], b2s[:])
                nc.gpsimd.dma_start(
                    outT[:, c0 + n * FD : c0 + (n + 1) * FD], ot[:]
                )

    nc.compile()
    nc.finalize()
    return nc


def _build_nc_raw():
    """Hand-scheduled version (no TileContext): explicit per-engine streams
    and semaphores. Avoids Tile's prologue/epilogue barriers (~13us fixed)."""
    mm_dt, _ = _dtypes()
    f32 = mybir.dt.float32
    nc = bacc.Bacc(
        "TRN2",
        target_bir_lowering=False,
        debug=False,
        num_devices=N_CORES,
    )

    # x arrives pre-arranged on the host as exactly the DMA pieces the
    # kernel issues: xp[g, h, c] is one fully CONTIGUOUS 0.5MB block with
    # 8KB rows = [k=2g | k=2g+1] columns of batch-half h for partition
    # chunk c (rows c*64..). Contiguous source + 8KB rows keeps the HWDGE
    # descriptor pipeline at full rate; the h-ordering lets phase-0
    # compute start long before all of x is resident. k=6 tail separate.
    NG = 3
    xp = nc.dram_tensor(
        "xp", [NG, NHALF, P, 2 * HB], mm_dt, kind="ExternalInput"
    )
    x6 = nc.dram_tensor("x6", [NHALF, KT[6], HB], mm_dt, kind="ExternalInput")
    w1 = nc.dram_tensor("w1t", [P, NK * HID], mm_dt, kind="ExternalInput")
    w2 = nc.dram_tensor("w2t", [P, 2 * NCLS], mm_dt, kind="ExternalInput")
    b1 = nc.dram_tensor("b1t", [P, 2], f32, kind="ExternalInput")
    b2 = nc.dram_tensor("b2t", [NCLS, 1], f32, kind="ExternalInput")
    outT = nc.dram_tensor("outT", [NCLS, BS], f32, kind="ExternalOutput")

    NB = NHALF * NT  # output column blocks of FD

    # SBUF: one tensor per k-tile pair (+ the k=6 tail); pair tensor g holds
    # [k2g-h0 | k2g+1-h0 | k2g-h1 | k2g+1-h1] as 2048-column panels.
    xps = [nc.alloc_sbuf_tensor(f"xp{g}", [P, 2 * BS], mm_dt).ap() for g in range(NG)]
    x6s = nc.alloc_sbuf_tensor("x6s", [KT[6], BS], mm_dt).ap()

    def x_rhs(h, k, n):
        """rhs AP for FC1 matmul (phase h, contraction tile k, n-tile n)."""
        if k == NK - 1:
            return x6s[0 : KT[6], h * HB + n * FD : h * HB + (n + 1) * FD]
        g, half = divmod(k, 2)
        c = (h * 2 + half) * HB + n * FD
        return xps[g][:, c : c + FD]
    w1s = nc.alloc_sbuf_tensor("w1s", [P, NK * HID], mm_dt).ap()
    w2s = nc.alloc_sbuf_tensor("w2s", [P, 2 * NCLS], mm_dt).ap()
    b1s = nc.alloc_sbuf_tensor("b1s", [P, 2], f32).ap()
    b2s = nc.alloc_sbuf_tensor("b2s", [NCLS, 1], f32).ap()
    h1a = nc.alloc_sbuf_tensor("h1a", [MT[0], BS], mm_dt).ap()
    h1b = nc.alloc_sbuf_tensor("h1b", [MT[1], BS], mm_dt).ap()
    ot = nc.alloc_sbuf_tensor("ot", [NCLS, BS], f32).ap()

    # PSUM: 2m x NT tensors covering all 8 banks; FC2 reuses them.
    ps = [
        [nc.alloc_psum_tensor(f"ps_{m}_{n}", [MT[m], FD], f32).ap() for n in range(NT)]
        for m in range(2)
    ]
    ps_flat = [ps[0][n] for n in range(NT)] + [ps[1][n] for n in range(NT)]

    # Each x k-tile is fetched as 4 pieces: partition chunk A (rows 0:64,
    # sync HWDGE ring) / B (rows 64:, scalar ring) x column half h. The
    # column halves let phase-0 compute start long before all of x is in.
    LO = 64

    if True:
        # One sem per transfer (completions across a queue are unordered).
        # alloc_semaphore (not the ctx-manager nc.semaphore) — the ctx exit
        # emits a per-sem clear instruction; we do one range-clear instead.
        s_x = [
            [nc.alloc_semaphore(f"s_x_{h}_{g}") for g in range(NG)]
            for h in range(NHALF)
        ]
        s_x6 = [nc.alloc_semaphore(f"s_x6_{h}") for h in range(NHALF)]
        s_w1a = nc.alloc_semaphore("s_w1a")
        s_w1a2 = nc.alloc_semaphore("s_w1a2")
        s_b1 = nc.alloc_semaphore("s_b1")
        s_b2 = nc.alloc_semaphore("s_b2")
        s_w2 = nc.alloc_semaphore("s_w2")
        s_mm = nc.alloc_semaphore("s_mm")
        s_da = nc.alloc_semaphore("s_da")
        s_dv = nc.alloc_semaphore("s_dv")
        s_f2 = nc.alloc_semaphore("s_f2")
        s_out = nc.alloc_semaphore("s_out")
        s_out2 = nc.alloc_semaphore("s_out2")
        all_sems = (
            [s for h in s_x for s in h]
            + s_x6
            + [s_w1a, s_w1a2, s_b1, s_b2, s_w2, s_mm, s_da,
               s_dv, s_f2, s_out, s_out2]
        )

        # closer index (1-based s_mm value) of the last matmul into bank (h,m,n)
        def closer(h, m, n):
            return h * 2 * NT + m * NT + n + 1

        with nc.Block(no_gpsimd_drain=True) as block:

            @block.sync
            def _(sync):
                # k=0/1 weight slices first so the first matmuls start early;
                # the rest of w1 follows the first x piece. Every x piece
                # spans all 128 partitions (keeps the SDMA engines
                # port-aligned and the ring at full rate).
                sync.dma_start(w1s[:, 0 : 2 * HID], w1[:, 0 : 2 * HID]).then_inc(
                    s_w1a, 16
                )
                for h in range(NHALF):
                    c0 = h * 2 * HB
                    for g in range(NG):
                        sync.dma_start(
                            xps[g][:, c0 : c0 + 2 * HB],
                            xp[g, h],
                        ).then_inc(s_x[h][g], 16)
                        if h == 0 and g == 0:
                            sync.dma_start(
                                w1s[:, 2 * HID :], w1[:, 2 * HID :]
                            ).then_inc(s_w1a2, 16)
                    sync.dma_start(
                        x6s[:, h * HB : (h + 1) * HB],
                        x6[h],
                    ).then_inc(s_x6[h], 16)
                # second half of the output overlaps the first (other ring)
                sync.wait_ge(s_f2, NB)
                sync.dma_start(
                    outT[:, BS // 2 :], ot[:, BS // 2 :]
                ).then_inc(s_out, 16)
                sync.wait_ge(s_out, 16)

            @block.scalar
            def _(scalar):
                scalar.dma_start(b1s[:], b1[:]).then_inc(s_b1, 16)
                scalar.dma_start(b2s[:], b2[:]).then_inc(s_b2, 16)
                scalar.dma_start(w2s[:], w2[:]).then_inc(s_w2, 16)
                # FC1 m0 drains: relu + bias from PSUM -> h1a (bf16 cast)
                scalar.wait_ge(s_b1, 16)  # b1s loaded
                for h in range(NHALF):
                    for n in range(NT):
                        scalar.wait_ge(s_mm, closer(h, 0, n))
                        c = h * HB + n * FD
                        nc.scalar.activation(
                            h1a[:, c : c + FD],
                            ps[0][n][:],
                            mybir.ActivationFunctionType.Relu,
                            bias=b1s[0 : MT[0], 0:1],
                        ).then_inc(s_da, 1)
                # first half of the output, overlapping FC2's second half
                scalar.wait_ge(s_f2, NB // 2)
                scalar.dma_start(
                    outT[:, : BS // 2], ot[:, : BS // 2]
                ).then_inc(s_out2, 16)
                scalar.wait_ge(s_out2, 16)

            @block.tensor
            def _(tensor):
                tensor.wait_ge(s_w1a, 16)
                for h in range(NHALF):
                    for k in range(NK):
                        kt = KT[k]
                        if k == 2:
                            # remainder of w1 (k>=2 column blocks)
                            tensor.wait_ge(s_w1a2, 16)
                        if k == NK - 1:
                            tensor.wait_ge(s_x6[h], 16)
                        elif k % 2 == 0:
                            tensor.wait_ge(s_x[h][k // 2], 16)
                        for m in range(2):
                            lhsT = w1s[0:kt, k * HID + m * P : k * HID + m * P + MT[m]]
                            for n in range(NT):
                                if h == 1 and k == 0:
                                    # bank reuse: wait for phase-0 drain (WAR)
                                    if m == 0:
                                        tensor.wait_ge(s_da, n + 1)
                                    else:
                                        tensor.wait_ge(s_dv, n + 1)
                                mm = nc.tensor.matmul(
                                    ps[m][n][:],
                                    lhsT,
                                    x_rhs(h, k, n),
                                    start=(k == 0),
                                    stop=(k == NK - 1),
                                )
                                if k == NK - 1:
                                    mm.then_inc(s_mm, 1)
                # FC2: block b covers columns [b*FD, (b+1)*FD), reuses ps_flat[b]
                tensor.wait_ge(s_w2, 16)  # w2s loaded
                for b in range(NB):
                    h, n = divmod(b, NT)
                    # bank ps_flat[b] free after its phase-1 FC1 drain
                    if b < NT:
                        tensor.wait_ge(s_da, NT + n + 1)
                    else:
                        tensor.wait_ge(s_dv, NT + n + 1)
                    # h1 slices for this block drained
                    tensor.wait_ge(s_da, b + 1)
                    tensor.wait_ge(s_dv, b + 1)
                    c = b * FD
                    nc.tensor.matmul(
                        ps_flat[b][0:NCLS, :],
                        w2s[0 : MT[0], 0:NCLS],
                        h1a[:, c : c + FD],
                        start=True,
                        stop=False,
                    )
                    nc.tensor.matmul(
                        ps_flat[b][0:NCLS, :],
                        w2s[0 : MT[1], NCLS : 2 * NCLS],
                        h1b[:, c : c + FD],
                        start=False,
                        stop=True,
                    ).then_inc(s_mm, 1)

            @block.vector
            def _(vector):
                vector.wait_ge(s_b1, 16)
                vector.wait_ge(s_b2, 16)
                # FC1 m1 drains: (psum + b1) max 0 -> h1b (bf16 cast)
                for h in range(NHALF):
                    for n in range(NT):
                        vector.wait_ge(s_mm, closer(h, 1, n))
                        c = h * HB + n * FD
                        nc.vector.tensor_scalar(
                            h1b[:, c : c + FD],
                            ps[1][n][:],
                            b1s[0 : MT[1], 1:2],
                            0.0,
                            mybir.AluOpType.add,
                            mybir.AluOpType.max,
                        ).then_inc(s_dv, 1)
                # FC2 drains: psum + b2 -> ot (fp32)
                for b in range(NB):
                    vector.wait_ge(s_mm, 2 * 2 * NT + b + 1)
                    nc.vector.tensor_scalar_add(
                        ot[:, b * FD : (b + 1) * FD], ps_flat[b][0:NCLS, :], b2s[:]
                    ).then_inc(s_f2, 1)

        # After the block-exit all-engine barrier every engine is synced;
        # reset sems so a re-execution of the NEFF starts clean.
        ids = sorted(s.num for s in all_sems)
        if ids == list(range(ids[0], ids[-1] + 1)):
            nc.gpsimd.sem_clear(range(ids[0], ids[-1] + 1))
        else:
            for s in all_sems:
                nc.gpsimd.sem_clear(s)

    nc.compile()
    nc.finalize()
    return nc


def _fold_weights(conv_w, W1):
    """W1eff[784,200] such that x @ W1eff == conv2d_valid(x, conv_w).flat @ W1.T"""
    W1r = W1.reshape(HID, OUTW, OUTW).transpose(1, 2, 0)  # [26,26,200]
    w1e = np.zeros((IMG, IMG, HID), np.float32)
    for di in range(KSZ):
        for dj in range(KSZ):
            w1e[di : di + OUTW, dj : dj + OUTW, :] += conv_w[di, dj] * W1r
    return w1e.reshape(NPIX, HID)


def _prepare_maps(x, conv_w, W1, b1, W2, b2, impl="raw"):
    _, np_dt = _dtypes()
    x = np.asarray(x, np.float32)
    conv_w = np.asarray(conv_w, np.float32)
    W1 = np.asarray(W1, np.float32)
    b1 = np.asarray(b1, np.float32)
    W2 = np.asarray(W2, np.float32)
    b2 = np.asarray(b2, np.float32)

    w1e = _fold_weights(conv_w, W1)
    w1t = np.zeros((P, NK * HID), np_dt)
    for k in range(NK):
        kt = KT[k]
        w1t[:kt, k * HID : (k + 1) * HID] = w1e[k * P : k * P + kt, :].astype(np_dt)
    W2T = W2.T  # [200, 10]
    w2t = np.zeros((P, 2 * NCLS), np_dt)
    w2t[: MT[0], 0:NCLS] = W2T[:P].astype(np_dt)
    w2t[: MT[1], NCLS : 2 * NCLS] = W2T[P:].astype(np_dt)
    b1t = np.zeros((P, 2), np.float32)
    b1t[: MT[0], 0] = b1[:P]
    b1t[: MT[1], 1] = b1[P:]
    b2t = b2.reshape(NCLS, 1)

    xs = x.reshape(N_CORES, BS, NPIX)
    maps = []
    for i in range(N_CORES):
        xTi = xs[i].T.astype(np_dt)  # [784, 4096]
        m = {"w1t": w1t, "w2t": w2t, "b1t": b1t, "b2t": b2t}
        if impl == "raw":
            # exact DMA-piece layout: [group, col-half, partition-chunk]
            # each piece [64, 2*HB] contiguous with 8KB rows [k2g | k2g+1]
            xpv = np.empty((3, NHALF, P, 2 * HB), np_dt)
            for g in range(3):
                r = g * 2 * P
                for h in range(NHALF):
                    cl, ch = h * HB, (h + 1) * HB
                    xpv[g, h, :, 0:HB] = xTi[r : r + P, cl:ch]
                    xpv[g, h, :, HB:] = xTi[r + P : r + 2 * P, cl:ch]
            m["xp"] = xpv
            x6v = np.empty((NHALF, NPIX - 6 * P, HB), np_dt)
            for h in range(NHALF):
                x6v[h] = xTi[6 * P :, h * HB : (h + 1) * HB]
            m["x6"] = x6v
        else:
            m["xT"] = xTi
        maps.append(m)
    return maps


def _run(inputs, trace=False):
    _ensure_axon_hooks()
    impl = os.environ.get("KERNEL_IMPL", "raw")
    key = ("nc", impl, bool(os.environ.get("KERNEL_FP32")))
    if key not in _cache:
        _cache[key] = _build_nc_raw() if impl == "raw" else _build_nc()
    nc = _cache[key]
    in_maps = _prepare_maps(**inputs, impl=impl)
    res = run_bass_kernel_spmd(nc, in_maps, list(range(N_CORES)), trace=trace)
    out = np.concatenate([r["outT"].T for r in res.results], axis=0)
    return out, res


def kernel(**inputs):
    out, _ = _run(inputs, trace=False)
    return out


# revision 44
# speedup vs baseline: 1.3388x; 1.0587x over previous
"""Trainium2 kernel for nn_DigitConvolutionalModel (dense_cnn).

Model: x[B,784] -> 3x3 valid conv (single channel) -> flatten[676]
       -> Linear(676,200) + ReLU -> Linear(200,10).

The conv is linear, so it is folded into the first Linear on the host:
  flat = x @ C  (C [784,676] sparse conv matrix)
  h1   = relu(flat @ W1.T + b1) = relu(x @ (C @ W1.T) + b1)
so the device computes a plain 784 -> 200 -> 10 MLP. Pure data
parallelism: batch 32768 is split into 8 shards of 4096, one per core;
weights are replicated. Each core receives x pre-transposed ([784,4096],
pixel on the partition/contraction axis) so both matmuls need no
on-device transpose:
  FC1: h1T[200,b] = W1eff[784,200].T @ xT[784,b]   (lhsT = W1eff)
  FC2: outT[10,b] = W2T[200,10].T  @ h1T[200,b]    (lhsT = W2.T)
"""

import os
import numpy as np
from contextlib import ExitStack

import concourse.bass as bass
import concourse.bacc as bacc
import concourse.mybir as mybir
import concourse.tile as tile
from concourse.bass_utils import run_bass_kernel_spmd

import ml_dtypes

N_CORES = 8
B = 32768
BS = B // N_CORES          # 4096 rows per core
IMG = 28
KSZ = 3
OUTW = IMG - KSZ + 1       # 26
NPIX = IMG * IMG           # 784
HID = 200
NCLS = 10

P = 128                    # SBUF partitions
LO_C = 64                  # partition chunk per HWDGE ring
FD = 512                   # matmul free dim (ISA max moving elements; 1 PSUM bank)
NK = 7                     # contraction tiles over 784 = 6*128 + 16
KT = [P] * 6 + [NPIX - 6 * P]
MT = [P, HID - P]          # hid output tiles: 128 + 72
NHALF = 2                  # batch halves per core (PSUM: 2m x 4n = 8 banks)
HB = BS // NHALF           # 2048
NT = HB // FD              # 4 n-tiles of 512 per half

_cache: dict = {}


def _ensure_axon_hooks():
    """Provide antenv.axon_hooks if the image lacks it.

    bass_utils' trace path does `from antenv.axon_hooks import
    get_axon_ntff_profile_hook`; on images without that module the import
    crashes instead of degrading. Register a minimal equivalent that drives
    NTFF profiling via the documented C ABI of the loaded axon PJRT plugin
    (axon_start_nrt_profile / axon_stop_nrt_profile), or returns None so
    bass_utils skips tracing gracefully.
    """
    try:
        import antenv.axon_hooks  # noqa: F401

        return
    except ImportError:
        pass
    import sys
    import types
    import ctypes
    import contextlib

    try:
        import antenv
    except ImportError:
        antenv = types.ModuleType("antenv")
        sys.modules["antenv"] = antenv

    mod = types.ModuleType("antenv.axon_hooks")
    state = {"hook": None, "built": False}

    def _build():
        so_path = None
        try:
            with open("/proc/self/maps") as f:
                for line in f:
                    if "libaxon_pjrt.so" in line:
                        so_path = line.split()[-1]
                        break
        except OSError:
            return None
        if so_path is None:
            return None
        lib = ctypes.CDLL(so_path)
        if not hasattr(lib, "axon_start_nrt_profile"):
            return None
        lib.axon_start_nrt_profile.argtypes = [
            ctypes.POINTER(ctypes.c_int64),
            ctypes.c_size_t,
        ]
        lib.axon_start_nrt_profile.restype = ctypes.c_int64
        lib.axon_stop_nrt_profile.argtypes = [ctypes.c_char_p]
        lib.axon_stop_nrt_profile.restype = ctypes.c_int64

        @contextlib.contextmanager
        def _hook(output_dir, device_ids):
            import jax

            jax.devices()
            if device_ids:
                ids = (ctypes.c_int64 * len(device_ids))(*device_ids)
                rc = lib.axon_start_nrt_profile(ids, len(device_ids))
            else:
                rc = lib.axon_start_nrt_profile(None, 0)
            if rc != 0:
                raise RuntimeError(f"axon_start_nrt_profile rc={rc}")
            try:
                yield
            finally:
                n = lib.axon_stop_nrt_profile(str(output_dir).encode())
                if n <= 0:
                    print(f"ntff profile: rc={n} (no profile written)")

        return _hook

    def get_axon_ntff_profile_hook():
        if not state["built"]:
            state["hook"] = _build()
            state["built"] = True
        return state["hook"]

    def set_axon_ntff_profile_hook(hook):
        state["hook"] = hook
        state["built"] = True

    mod.get_axon_ntff_profile_hook = get_axon_ntff_profile_hook
    mod.set_axon_ntff_profile_hook = set_axon_ntff_profile_hook
    sys.modules["antenv.axon_hooks"] = mod
    antenv.axon_hooks = mod


def _dtypes():
    if os.environ.get("KERNEL_FP32"):
        return mybir.dt.float32, np.float32
    return mybir.dt.bfloat16, ml_dtypes.bfloat16


def _build_nc():
    mm_dt, _ = _dtypes()
    f32 = mybir.dt.float32
    # Bacc (not plain Bass): its compile() pass splits multi-sem waits into
    # standalone EventSemaphore instructions — the TPB ISA allows only one
    # embedded wait per instruction.
    nc = bacc.Bacc(
        "TRN2",
        target_bir_lowering=False,
        debug=False,
        num_devices=N_CORES,
    )

    xT = nc.dram_tensor("xT", [NPIX, BS], mm_dt, kind="ExternalInput")
    w1 = nc.dram_tensor("w1t", [P, NK * HID], mm_dt, kind="ExternalInput")
    w2 = nc.dram_tensor("w2t", [P, 2 * NCLS], mm_dt, kind="ExternalInput")
    b1 = nc.dram_tensor("b1t", [P, 2], f32, kind="ExternalInput")
    b2 = nc.dram_tensor("b2t", [NCLS, 1], f32, kind="ExternalInput")
    outT = nc.dram_tensor("outT", [NCLS, BS], f32, kind="ExternalOutput")

    with ExitStack() as ctx:
        tc = ctx.enter_context(tile.TileContext(nc))
        const = ctx.enter_context(tc.tile_pool(name="const", bufs=1))
        xp = ctx.enter_context(tc.tile_pool(name="xp", bufs=NHALF * NK))
        h1p = ctx.enter_context(tc.tile_pool(name="h1p", bufs=2))
        op = ctx.enter_context(tc.tile_pool(name="op", bufs=NHALF * NT))
        pp = ctx.enter_context(tc.tile_pool(name="pp", bufs=8, space="PSUM"))

        w1s = const.tile([P, NK * HID], mm_dt)
        w2s = const.tile([P, 2 * NCLS], mm_dt)
        b1s = const.tile([P, 2], f32)
        b2s = const.tile([NCLS, 1], f32)
        nc.sync.dma_start(w1s[:], w1[:])
        nc.sync.dma_start(w2s[:], w2[:])
        nc.sync.dma_start(b1s[:], b1[:])
        nc.sync.dma_start(b2s[:], b2[:])

        h1tiles = []
        for h in range(NHALF):
            c0 = h * HB
            ps = [
                [
                    pp.tile([MT[m], FD], f32, tag="bank", name=f"ps_{h}_{m}_{n}")
                    for n in range(NT)
                ]
                for m in range(2)
            ]
            for k in range(NK):
                kt = KT[k]
                xt = xp.tile([P, HB], mm_dt, tag="xt", name=f"xt_{h}_{k}")
                nc.sync.dma_start(xt[:kt, :], xT[k * P : k * P + kt, c0 : c0 + HB])
                for m in range(2):
                    lhsT = w1s[0:kt, k * HID + m * P : k * HID + m * P + MT[m]]
                    for n in range(NT):
                        nc.tensor.matmul(
                            ps[m][n][:],
                            lhsT,
                            xt[0:kt, n * FD : (n + 1) * FD],
                            start=(k == 0),
                            stop=(k == NK - 1),
                        )
            h1 = [
                h1p.tile([MT[0], HB], mm_dt, tag="h1a", name=f"h1a_{h}"),
                h1p.tile([MT[1], HB], mm_dt, tag="h1b", name=f"h1b_{h}"),
            ]
            # Drains split across ACT (m0, relu via LUT with bias) and DVE
            # (m1, add-bias then max-0) so the banks free twice as fast.
            for n in range(NT):
                nc.scalar.activation(
                    h1[0][:, n * FD : (n + 1) * FD],
                    ps[0][n][:],
                    mybir.ActivationFunctionType.Relu,
                    bias=b1s[0 : MT[0], 0:1],
                )
            for n in range(NT):
                nc.vector.tensor_scalar(
                    h1[1][:, n * FD : (n + 1) * FD],
                    ps[1][n][:],
                    b1s[0 : MT[1], 1:2],
                    0.0,
                    mybir.AluOpType.add,
                    mybir.AluOpType.max,
                )
            h1tiles.append(h1)

        # FC2 emitted after all FC1 matmuls: PE stays dense through FC1,
        # FC2 runs at the tail when h1 has long been drained.
        for h in range(NHALF):
            c0 = h * HB
            h1 = h1tiles[h]
            for n in range(NT):
                ps2 = pp.tile([NCLS, FD], f32, tag="bank", name=f"ps2_{h}_{n}")
                for m in range(2):
                    nc.tensor.matmul(
                        ps2[:],
                        w2s[0 : MT[m], m * NCLS : (m + 1) * NCLS],
                        h1[m][:, n * FD : (n + 1) * FD],
                        start=(m == 0),
                        stop=(m == 1),
                    )
                ot = op.tile([NCLS, FD], f32, tag="ot", name=f"ot_{h}_{n}")
                nc.vector.tensor_scalar_add(ot[:], ps2[:], b2s[:])
                nc.gpsimd.dma_start(
                    outT[:, c0 + n * FD : c0 + (n + 1) * FD], ot[:]
                )

    nc.compile()
    nc.finalize()
    return nc


def _build_nc_raw():
    """Hand-scheduled version (no TileContext): explicit per-engine streams
    and semaphores. Avoids Tile's prologue/epilogue barriers (~13us fixed)."""
    mm_dt, _ = _dtypes()
    f32 = mybir.dt.float32
    nc = bacc.Bacc(
        "TRN2",
        target_bir_lowering=False,
        debug=False,
        num_devices=N_CORES,
    )

    xT = nc.dram_tensor("xT", [NPIX, BS], mm_dt, kind="ExternalInput")
    w1 = nc.dram_tensor("w1t", [P, NK * HID], mm_dt, kind="ExternalInput")
    w2 = nc.dram_tensor("w2t", [P, 2 * NCLS], mm_dt, kind="ExternalInput")
    b1 = nc.dram_tensor("b1t", [P, 2], f32, kind="ExternalInput")
    b2 = nc.dram_tensor("b2t", [NCLS, 1], f32, kind="ExternalInput")
    outT = nc.dram_tensor("outT", [NCLS, BS], f32, kind="ExternalOutput")

    NB = NHALF * NT  # output column blocks of FD

    # SBUF
    xts = [nc.alloc_sbuf_tensor(f"xt{k}", [P, BS], mm_dt).ap() for k in range(NK)]
    w1s = nc.alloc_sbuf_tensor("w1s", [P, NK * HID], mm_dt).ap()
    w2s = nc.alloc_sbuf_tensor("w2s", [P, 2 * NCLS], mm_dt).ap()
    b1s = nc.alloc_sbuf_tensor("b1s", [P, 2], f32).ap()
    b2s = nc.alloc_sbuf_tensor("b2s", [NCLS, 1], f32).ap()
    h1a = nc.alloc_sbuf_tensor("h1a", [MT[0], BS], mm_dt).ap()
    h1b = nc.alloc_sbuf_tensor("h1b", [MT[1], BS], mm_dt).ap()
    ot = nc.alloc_sbuf_tensor("ot", [NCLS, BS], f32).ap()

    # PSUM: 2m x NT tensors covering all 8 banks; FC2 reuses them.
    ps = [
        [nc.alloc_psum_tensor(f"ps_{m}_{n}", [MT[m], FD], f32).ap() for n in range(NT)]
        for m in range(2)
    ]
    ps_flat = [ps[0][n] for n in range(NT)] + [ps[1][n] for n in range(NT)]

    if True:
        # One sem per transfer (completions across a queue are unordered).
        # alloc_semaphore (not the ctx-manager nc.semaphore) — the ctx exit
        # emits a per-sem clear instruction; we do one range-clear instead.
        s_x = [
            [nc.alloc_semaphore(f"s_x_{h}_{k}") for k in range(NK)]
            for h in range(NHALF)
        ]
        s_w1a = nc.alloc_semaphore("s_w1a")
        s_w1a2 = nc.alloc_semaphore("s_w1a2")
        s_b1 = nc.alloc_semaphore("s_b1")
        s_b2 = nc.alloc_semaphore("s_b2")
        s_w2 = nc.alloc_semaphore("s_w2")
        s_mm = nc.alloc_semaphore("s_mm")
        s_da = nc.alloc_semaphore("s_da")
        s_dv = nc.alloc_semaphore("s_dv")
        s_f2a = nc.alloc_semaphore("s_f2a")
        s_f2b = nc.alloc_semaphore("s_f2b")
        s_out = nc.alloc_semaphore("s_out")
        s_out2 = nc.alloc_semaphore("s_out2")
        all_sems = (
            [s for h in s_x for s in h]
            + [s_w1a, s_w1a2, s_b1, s_b2, s_w2, s_mm, s_da, s_dv,
               s_f2a, s_f2b, s_out, s_out2]
        )

        # closer index (1-based s_mm value) of the last matmul into bank (h,m,n)
        def closer(h, m, n):
            return h * 2 * NT + m * NT + n + 1

        with nc.Block(no_gpsimd_drain=True) as block:

            @block.sync
            def _(sync):
                # all x on this ring; every piece spans 128 partitions so the
                # SDMA engines stay port-aligned (full ring rate). k=0/1
                # weight slice first so the first matmuls can start early.
                sync.dma_start(w1s[:, 0 : 2 * HID], w1[:, 0 : 2 * HID]).then_inc(
                    s_w1a, 16
                )
                for h in range(NHALF):
                    c0 = h * HB
                    for k in range(NK):
                        kt = KT[k]
                        sync.dma_start(
                            xts[k][0:kt, c0 : c0 + HB],
                            xT[k * P : k * P + kt, c0 : c0 + HB],
                        ).then_inc(s_x[h][k], 16)
                        if h == 0 and k == 0:
                            sync.dma_start(
                                w1s[:, 2 * HID :], w1[:, 2 * HID :]
                            ).then_inc(s_w1a2, 16)
                # second half of the output overlaps the first (other ring)
                sync.wait_ge(s_f2a, NB // 2)
                sync.wait_ge(s_f2b, NB // 2)
                sync.dma_start(
                    outT[:, BS // 2 :], ot[:, BS // 2 :]
                ).then_inc(s_out, 16)
                sync.wait_ge(s_out, 16)

            @block.scalar
            def _(scalar):
                scalar.dma_start(b1s[:], b1[:]).then_inc(s_b1, 16)
                scalar.dma_start(b2s[:], b2[:]).then_inc(s_b2, 16)
                scalar.dma_start(w2s[:], w2[:]).then_inc(s_w2, 16)
                # FC1 m0 drains: relu + bias from PSUM -> h1a (bf16 cast)
                scalar.wait_ge(s_b1, 16)
                for h in range(NHALF):
                    for n in range(NT):
                        scalar.wait_ge(s_mm, closer(h, 0, n))
                        c = h * HB + n * FD
                        nc.scalar.activation(
                            h1a[:, c : c + FD],
                            ps[0][n][:],
                            mybir.ActivationFunctionType.Relu,
                            bias=b1s[0 : MT[0], 0:1],
                        ).then_inc(s_da, 1)
                # FC2 drains for even blocks (ACT side), b2 bias + identity
                scalar.wait_ge(s_b2, 16)
                for b in range(0, NB, 2):
                    scalar.wait_ge(s_mm, 2 * 2 * NT + b + 1)
                    nc.scalar.activation(
                        ot[:, b * FD : (b + 1) * FD],
                        ps_flat[b][0:NCLS, :],
                        mybir.ActivationFunctionType.Identity,
                        bias=b2s[:],
                    ).then_inc(s_f2a, 1)
                # first half of the output, overlapping FC2's second half
                scalar.wait_ge(s_f2a, NB // 4)
                scalar.wait_ge(s_f2b, NB // 4)
                scalar.dma_start(
                    outT[:, : BS // 2], ot[:, : BS // 2]
                ).then_inc(s_out2, 16)
                scalar.wait_ge(s_out2, 16)

            @block.tensor
            def _(tensor):
                tensor.wait_ge(s_w1a, 16)
                for h in range(NHALF):
                    for k in range(NK):
                        kt = KT[k]
                        if k == 2:
                            # remainder of w1 (k>=2 column blocks)
                            tensor.wait_ge(s_w1a2, 16)
                        tensor.wait_ge(s_x[h][k], 16)
                        for m in range(2):
                            lhsT = w1s[0:kt, k * HID + m * P : k * HID + m * P + MT[m]]
                            for n in range(NT):
                                if h == 1 and k == 0:
                                    # bank reuse: wait for phase-0 drain (WAR)
                                    if m == 0:
                                        tensor.wait_ge(s_da, n + 1)
                                    else:
                                        tensor.wait_ge(s_dv, n + 1)
                                mm = nc.tensor.matmul(
                                    ps[m][n][:],
                                    lhsT,
                                    xts[k][0:kt, h * HB + n * FD : h * HB + (n + 1) * FD],
                                    start=(k == 0),
                                    stop=(k == NK - 1),
                                )
                                if k == NK - 1:
                                    mm.then_inc(s_mm, 1)
                # FC2: block b covers columns [b*FD, (b+1)*FD), reuses ps_flat[b]
                tensor.wait_ge(s_w2, 16)
                for b in range(NB):
                    h, n = divmod(b, NT)
                    # bank ps_flat[b] free after its phase-1 FC1 drain
                    if b < NT:
                        tensor.wait_ge(s_da, NT + n + 1)
                    else:
                        tensor.wait_ge(s_dv, NT + n + 1)
                    # h1 slices for this block drained
                    tensor.wait_ge(s_da, b + 1)
                    tensor.wait_ge(s_dv, b + 1)
                    c = b * FD
                    nc.tensor.matmul(
                        ps_flat[b][0:NCLS, :],
                        w2s[0 : MT[0], 0:NCLS],
                        h1a[:, c : c + FD],
                        start=True,
                        stop=False,
                    )
                    nc.tensor.matmul(
                        ps_flat[b][0:NCLS, :],
                        w2s[0 : MT[1], NCLS : 2 * NCLS],
                        h1b[:, c : c + FD],
                        start=False,
                        stop=True,
                    ).then_inc(s_mm, 1)

            @block.vector
            def _(vector):
                vector.wait_ge(s_b1, 16)
                vector.wait_ge(s_b2, 16)
                # FC1 m1 drains: (psum + b1) max 0 -> h1b (bf16 cast)
                for h in range(NHALF):
                    for n in range(NT):
                        vector.wait_ge(s_mm, closer(h, 1, n))
                        c = h * HB + n * FD
                        nc.vector.tensor_scalar(
                            h1b[:, c : c + FD],
                            ps[1][n][:],
                            b1s[0 : MT[1], 1:2],
                            0.0,
                            mybir.AluOpType.add,
                            mybir.AluOpType.max,
                        ).then_inc(s_dv, 1)
                # FC2 drains for odd blocks (DVE side): psum + b2 -> ot
                for b in range(1, NB, 2):
                    vector.wait_ge(s_mm, 2 * 2 * NT + b + 1)
                    nc.vector.tensor_scalar_add(
                        ot[:, b * FD : (b + 1) * FD], ps_flat[b][0:NCLS, :], b2s[:]
                    ).then_inc(s_f2b, 1)

        # After the block-exit all-engine barrier every engine is synced;
        # reset sems so a re-execution of the NEFF starts clean.
        ids = sorted(s.num for s in all_sems)
        if ids == list(range(ids[0], ids[-1] + 1)):
            nc.gpsimd.sem_clear(range(ids[0], ids[-1] + 1))
        else:
            for s in all_sems:
                nc.gpsimd.sem_clear(s)

    nc.compile()
    nc.finalize()
    return nc


def _fold_weights(conv_w, W1):
    """W1eff[784,200] such that x @ W1eff == conv2d_valid(x, conv_w).flat @ W1.T"""
    W1r = W1.reshape(HID, OUTW, OUTW).transpose(1, 2, 0)  # [26,26,200]
    w1e = np.zeros((IMG, IMG, HID), np.float32)
    for di in range(KSZ):
        for dj in range(KSZ):
            w1e[di : di + OUTW, dj : dj + OUTW, :] += conv_w[di, dj] * W1r
    return w1e.reshape(NPIX, HID)


def _prepare_maps(x, conv_w, W1, b1, W2, b2, impl="raw"):
    _, np_dt = _dtypes()
    x = np.asarray(x, np.float32)
    conv_w = np.asarray(conv_w, np.float32)
    W1 = np.asarray(W1, np.float32)
    b1 = np.asarray(b1, np.float32)
    W2 = np.asarray(W2, np.float32)
    b2 = np.asarray(b2, np.float32)

    w1e = _fold_weights(conv_w, W1)
    w1t = np.zeros((P, NK * HID), np_dt)
    for k in range(NK):
        kt = KT[k]
        w1t[:kt, k * HID : (k + 1) * HID] = w1e[k * P : k * P + kt, :].astype(np_dt)
    W2T = W2.T  # [200, 10]
    w2t = np.zeros((P, 2 * NCLS), np_dt)
    w2t[: MT[0], 0:NCLS] = W2T[:P].astype(np_dt)
    w2t[: MT[1], NCLS : 2 * NCLS] = W2T[P:].astype(np_dt)
    b1t = np.zeros((P, 2), np.float32)
    b1t[: MT[0], 0] = b1[:P]
    b1t[: MT[1], 1] = b1[P:]
    b2t = b2.reshape(NCLS, 1)

    xs = x.reshape(N_CORES, BS, NPIX)
    maps = []
    for i in range(N_CORES):
        xTi = xs[i].T.astype(np_dt)  # [784, 4096]
        m = {"w1t": w1t, "w2t": w2t, "b1t": b1t, "b2t": b2t}
        m["xT"] = xTi
        maps.append(m)
    return maps


def _run(inputs, trace=False):
    _ensure_axon_hooks()
    impl = os.environ.get("KERNEL_IMPL", "raw")
    key = ("nc", impl, bool(os.environ.get("KERNEL_FP32")))
    if key not in _cache:
        _cache[key] = _build_nc_raw() if impl == "raw" else _build_nc()
    nc = _cache[key]
    in_maps = _prepare_maps(**inputs, impl=impl)
    res = run_bass_kernel_spmd(nc, in_maps, list(range(N_CORES)), trace=trace)
    out = np.concatenate([r["outT"].T for r in res.results], axis=0)
    return out, res


def kernel(**inputs):
    out, _ = _run(inputs, trace=False)
    return out
